# revision 13
# baseline (speedup 1.0000x reference)
"""Trainium2 Bass kernel for nn_MiniLLMIndexer.

Computes: q = hs @ wq.T, k = hs @ wk.T (per-head reshape), per-head scaled
attention scores, mean over heads, +mask pad, top-1024 indices (descending,
per query row).

Key algebraic fold: mean over heads of per-head dot products equals one
full-width dot product:
    mean_h(q_h . k_h) * scale = (hs@wq.T) . (hs@wk.T) * scale / NH
so scores_mean = qf @ kf.T * (scale/NH), qf/kf: [S, 256]. No per-head work.

Sharding: 4096 query rows split across 8 cores (512 rows each; cores 0-3
batch 0, cores 4-7 batch 1). Each core computes kf for its whole batch
locally -> no collectives.

Top-k: full bitonic sort (descending) of each 2048-wide score row carrying
(fp32 value, uint16 index), on the vector engine (min/max/copy_predicated)
with gpsimd doing mask generation + index staging copies. Final phase merges
only the top half. Indices of the first 1024 slots are the answer.
"""

import sys

if "/opt/trn_rl_repo" not in sys.path:
    sys.path.insert(0, "/opt/trn_rl_repo")

import numpy as np

from concourse import bacc, bass, mybir, tile
from concourse.bass_utils import run_bass_kernel_spmd

B, S, HID = 2, 2048, 1024
NH, HD = 8, 32
TOPK = 1024
NCORES = 8
ROWS_PER_CORE = (B * S) // NCORES  # 512
D = NH * HD  # 256
SCALE = (HD ** -0.5) / NH

F32 = mybir.dt.float32
U8 = mybir.dt.uint8
U16 = mybir.dt.uint16
I32 = mybir.dt.int32

_CACHE = {}


def _network_layers(n=S):
    """Bitonic network: descending sort via flip-merge. Returns list of
    (kind, param, width) where width limits processing to the first
    `width` elements (final merge only needs the top half)."""
    layers = []
    m = 1
    while 2 * m <= n:
        layers.append(("flip", m, n))
        d = m // 2
        width = n // 2 if 2 * m == n else n
        while d >= 1:
            layers.append(("dist", d, width))
            d //= 2
        m *= 2
    return layers


def _build_program():
    nc = bacc.Bacc(None, target_bir_lowering=False)

    hsT = nc.dram_tensor("hsT", [HID, S], F32, kind="ExternalInput")
    hsTo = nc.dram_tensor("hsTo", [HID, ROWS_PER_CORE], F32, kind="ExternalInput")
    wqT = nc.dram_tensor("wqT", [HID, D], F32, kind="ExternalInput")
    wkT = nc.dram_tensor("wkT", [HID, D], F32, kind="ExternalInput")
    maskd = nc.dram_tensor("maskd", [1, S], F32, kind="ExternalInput")
    out = nc.dram_tensor("out", [ROWS_PER_CORE, TOPK], I32, kind="ExternalOutput")

    HC = HID // 128  # 8 contraction chunks
    DC = D // 128    # 2 d-half chunks
    JC = S // 512    # 4 column chunks
    RT = ROWS_PER_CORE // 128  # 4 row tiles

    layers = _network_layers()

    with tile.TileContext(nc) as tc:
        with (
            tc.tile_pool(name="weights", bufs=1) as wpool,
            tc.tile_pool(name="kf", bufs=1) as kfpool,
            tc.tile_pool(name="psum", bufs=6, space="PSUM") as psum,
            tc.tile_pool(name="psum_d", bufs=1, space="PSUM") as psum_d,
            tc.tile_pool(name="small", bufs=1) as small,
            tc.tile_pool(name="hs", bufs=1) as hspool,
            tc.tile_pool(name="sort", bufs=2) as spool,
        ):
            # ---- load weights / mask ----
            wq_sb = wpool.tile([128, HC, D], F32, tag="wq")
            wk_sb = wpool.tile([128, HC, D], F32, tag="wk")
            nc.sync.dma_start(wq_sb[:], wqT.rearrange("(c p) f -> p c f", p=128))
            nc.sync.dma_start(wk_sb[:], wkT.rearrange("(c p) f -> p c f", p=128))

            pad_sb = small.tile([1, S], F32, tag="pad")
            nc.sync.dma_start(pad_sb[:], maskd[:])
            # pad = (1 - mask) * -1e9 = mask*1e9 - 1e9 (in place)
            nc.vector.tensor_scalar(
                pad_sb[:], pad_sb[:], 1e9, scalar2=1e9,
                op0=mybir.AluOpType.mult, op1=mybir.AluOpType.subtract,
            )
            ones_sb = small.tile([1, 128], F32, tag="ones")
            nc.vector.memset(ones_sb[:], 1.0)

            # tiny dummy matmuls so the PE queue observes the weight-DMA
            # semaphores before any real matmul (PE LDW has 1 wait slot)
            dummy_ps = psum_d.tile([1, 1], F32, tag="dummy")
            nc.tensor.matmul(dummy_ps[:], wq_sb[:, 0, 0:1], wq_sb[:, 0, 0:1])
            nc.tensor.matmul(dummy_ps[:], wk_sb[:, 0, 0:1], wk_sb[:, 0, 0:1])

            # ---- own-rows hs (transposed) for qf ----
            hso_sb = [hspool.tile([128, ROWS_PER_CORE], F32, name=f"hso{h}", tag=f"hso{h}")
                      for h in range(HC)]
            for h in range(HC):
                nc.sync.dma_start(
                    hso_sb[h][:],
                    hsTo.rearrange("(c p) f -> p c f", p=128)[:, h, :],
                )

            # qfT[d, i] (scaled): 2 tiles [128, 512]
            qf_sb = wpool.tile([128, DC, ROWS_PER_CORE], F32, tag="qf")
            for dh in range(DC):
                acc = psum.tile([128, ROWS_PER_CORE], F32, tag="ps")
                for h in range(HC):
                    nc.tensor.matmul(
                        acc[:],
                        wq_sb[:, h, dh * 128:(dh + 1) * 128],
                        hso_sb[h][:],
                        start=(h == 0), stop=(h == HC - 1),
                    )
                nc.scalar.activation(
                    qf_sb[:, dh, :], acc[:],
                    mybir.ActivationFunctionType.Copy, scale=float(SCALE),
                )

            # ---- full hs (transposed) for kf ----
            hs_sb = [hspool.tile([128, S], F32, name=f"hsfull{h}", tag=f"hsfull{h}")
                     for h in range(HC)]
            for h in range(HC):
                nc.sync.dma_start(
                    hs_sb[h][:],
                    hsT.rearrange("(c p) f -> p c f", p=128)[:, h, :],
                )

            # kfT[d, j]: 2 tiles [128, 2048]
            kf_sb = kfpool.tile([128, DC, S], F32, tag="kf")
            for dh in range(DC):
                for jc in range(JC):
                    acc = psum.tile([128, 512], F32, tag="ps")
                    for h in range(HC):
                        nc.tensor.matmul(
                            acc[:],
                            wk_sb[:, h, dh * 128:(dh + 1) * 128],
                            hs_sb[h][:, jc * 512:(jc + 1) * 512],
                            start=(h == 0), stop=(h == HC - 1),
                        )
                    nc.scalar.activation(
                        kf_sb[:, dh, jc * 512:(jc + 1) * 512], acc[:],
                        mybir.ActivationFunctionType.Copy,
                    )

            # ---- per row-tile: scores matmul + bitonic top-k ----
            for rt in range(RT):
                val_a = spool.tile([128, S], F32, tag="val_a")
                val_b = spool.tile([128, S], F32, tag="val_b")
                idx_a = spool.tile([128, S], U16, tag="idx_a")
                idx_b = spool.tile([128, S], U16, tag="idx_b")
                mask8_a = spool.tile([128, S // 2], U8, tag="mask8_a")
                mask8_b = spool.tile([128, S // 2], U8, tag="mask8_b")

                # scores[i, j] for i in this row tile -> val_a
                for jc in range(JC):
                    acc = psum.tile([128, 512], F32, tag="ps")
                    for dh in range(DC):
                        nc.tensor.matmul(
                            acc[:],
                            qf_sb[:, dh, rt * 128:(rt + 1) * 128],
                            kf_sb[:, dh, jc * 512:(jc + 1) * 512],
                            start=(dh == 0), stop=False,
                        )
                    # + pad broadcast along rows (rank-1 with ones)
                    nc.tensor.matmul(
                        acc[:],
                        ones_sb[:, :],
                        pad_sb[:, jc * 512:(jc + 1) * 512],
                        start=False, stop=True,
                    )
                    nc.scalar.activation(
                        val_a[:, jc * 512:(jc + 1) * 512], acc[:],
                        mybir.ActivationFunctionType.Copy,
                    )

                nc.gpsimd.iota(idx_a[:], pattern=[[1, S]], base=0,
                               channel_multiplier=0)

                cur_v, nxt_v = val_a, val_b
                cur_i, nxt_i = idx_a, idx_b
                cur_m8, nxt_m8 = mask8_a, mask8_b

                for (kind, param, width) in layers:
                    if kind == "flip":
                        m = param
                        nb = width // (2 * m)
                        vv = cur_v[:, :width].rearrange(
                            "p (nb two m) -> p nb two m", two=2, m=m)
                        nv = nxt_v[:, :width].rearrange(
                            "p (nb two m) -> p nb two m", two=2, m=m)
                        iv = cur_i[:, :width].rearrange(
                            "p (nb two m) -> p nb two m", two=2, m=m)
                        ni = nxt_i[:, :width].rearrange(
                            "p (nb two m) -> p nb two m", two=2, m=m)
                        a, b = vv[:, :, 0, :], vv[:, :, 1, ::-1]
                        na, nb_ = nv[:, :, 0, :], nv[:, :, 1, ::-1]
                        ia, ib = iv[:, :, 0, :], iv[:, :, 1, ::-1]
                        nia, nib = ni[:, :, 0, :], ni[:, :, 1, ::-1]
                        mk8 = cur_m8[:, : width // 2].rearrange(
                            "p (nb m) -> p nb m", m=m)
                    else:
                        d = param
                        nb = width // (2 * d)
                        vv = cur_v[:, :width].rearrange(
                            "p (nb two d) -> p nb two d", two=2, d=d)
                        nv = nxt_v[:, :width].rearrange(
                            "p (nb two d) -> p nb two d", two=2, d=d)
                        iv = cur_i[:, :width].rearrange(
                            "p (nb two d) -> p nb two d", two=2, d=d)
                        ni = nxt_i[:, :width].rearrange(
                            "p (nb two d) -> p nb two d", two=2, d=d)
                        a, b = vv[:, :, 0, :], vv[:, :, 1, :]
                        na, nb_ = nv[:, :, 0, :], nv[:, :, 1, :]
                        ia, ib = iv[:, :, 0, :], iv[:, :, 1, :]
                        nia, nib = ni[:, :, 0, :], ni[:, :, 1, :]
                        mk8 = cur_m8[:, : width // 2].rearrange(
                            "p (nb d) -> p nb d", d=d)

                    # mask = (a >= b) -> u8, on DVE
                    nc.vector.tensor_tensor(mk8, a, b, mybir.AluOpType.is_ge)
                    nc.gpsimd.tensor_copy(nia, ib)
                    nc.gpsimd.tensor_copy(nib, ia)
                    # values on DVE
                    nc.vector.tensor_tensor(na, a, b, mybir.AluOpType.max)
                    nc.vector.tensor_tensor(nb_, a, b, mybir.AluOpType.min)
                    # indices: overwrite with winner where mask
                    nc.vector.copy_predicated(nia, mk8, ia)
                    nc.vector.copy_predicated(nib, mk8, ib)

                    cur_v, nxt_v = nxt_v, cur_v
                    cur_i, nxt_i = nxt_i, cur_i
                    cur_m8, nxt_m8 = nxt_m8, cur_m8

                out_sb = spool.tile([128, TOPK], I32, tag="outi")
                nc.vector.tensor_copy(out_sb[:], cur_i[:, :TOPK])
                nc.sync.dma_start(out[rt * 128:(rt + 1) * 128, :], out_sb[:])
    if not nc.is_finalized():
        nc.finalize()
    return nc


def _get_program():
    if "nc" not in _CACHE:
        _CACHE["nc"] = _build_program()
    return _CACHE["nc"]


def kernel(hidden_states, attention_mask, wq, wk, past_len=0):
    hidden_states = np.asarray(hidden_states, dtype=np.float32)
    attention_mask = np.asarray(attention_mask, dtype=np.float32)
    wq = np.asarray(wq, dtype=np.float32)
    wk = np.asarray(wk, dtype=np.float32)

    nc = _get_program()

    wqT = np.ascontiguousarray(wq.T)
    wkT = np.ascontiguousarray(wk.T)
    hsT = [np.ascontiguousarray(hidden_states[b].T) for b in range(B)]

    in_maps = []
    for c in range(NCORES):
        b = c // (NCORES // B)
        r0 = (c % (NCORES // B)) * ROWS_PER_CORE
        in_maps.append({
            "hsT": hsT[b],
            "hsTo": np.ascontiguousarray(hsT[b][:, r0:r0 + ROWS_PER_CORE]),
            "wqT": wqT,
            "wkT": wkT,
            "maskd": attention_mask[b][None, :],
        })

    res = run_bass_kernel_spmd(nc, in_maps, core_ids=list(range(NCORES)))
    parts = [res.results[c]["out"] for c in range(NCORES)]
    full = np.concatenate(parts, axis=0).reshape(B, S, TOPK)
    return full.astype(np.int32)
